# revision 1
# baseline (speedup 1.0000x reference)
"""Complex-valued attention (nn_Attention_1) on 8 Trainium2 NeuronCores.

Math (per batch b):
  q = X @ Wq_cat, k = Y @ Wk_cat, v = Y @ Wv_cat  with X=[Q_r|Q_i], Y=[KV_r|KV_i]
  scores = Re(q k^T) + 2 qi ki^T  ==  sum_x X_x (Wq_cat Wk_cat^T) Y_x^T
  probs = softmax(scores + kmask_bias); ctx = probs @ v * Q_mask
Sharding: data-parallel over B=16 -> 2 batches per core, no cross-core comm.

Precision scheme: the softmax here is argmax-sharp (score std ~90), so scores
need fp32-grade accuracy or near-tie rows flip and blow up absmax error.
All matmuls run as fp32r (TF32, 1 cyc/row at N=512) with hi/lo splitting where
full precision matters:
  Z^T = (Mhi+Mlo).T (Xhi+Xlo)  3-pass   (exact to ~2^-22)
  S   = (Zhi+Zlo).T (Yhi+Ylo)  3-pass   (exact to ~2^-22)
  P   = exp(S - rowmax) rounded to tf32 (ACT)
  v   = Yhi.T Wv_bd   1-pass            (~3e-4 rel, fine)
  ctx = P^T.T v       1-pass tf32       (~3e-4 rel, no softmax amplification)
Normalization (1/sumexp) and Q_mask are fused into the ctx PSUM->SBUF copy.
"""
import sys
sys.path.insert(0, '/opt/trn_rl_repo')
import numpy as np
import ml_dtypes
from contextlib import ExitStack

import concourse.bass as bass
from concourse import bacc
import concourse.mybir as mybir
import concourse.tile as tile
from concourse.bass_utils import run_bass_kernel_spmd

B, S, E = 16, 512, 32
NCORES = 8
BPC = B // NCORES           # batches per core
NCH = 16                    # 128-row chunks of the 2048-wide (x, e-cat) axis
SQT = S // 128              # 4 sq tiles per batch

f32 = mybir.dt.float32
f32r = mybir.dt.float32r
bf16 = mybir.dt.bfloat16

LAST_EXEC_NS = None
_NC_CACHE = None


def build_nc():
    nc = bacc.Bacc()
    CW = NCH * 512
    xh = nc.dram_tensor("xh", [BPC, 128, CW], f32r, kind="ExternalInput")
    xl = nc.dram_tensor("xl", [BPC, 128, CW], bf16, kind="ExternalInput")
    yh = nc.dram_tensor("yh", [BPC, 128, CW], f32r, kind="ExternalInput")
    yl = nc.dram_tensor("yl", [BPC, 128, CW], bf16, kind="ExternalInput")
    mh = nc.dram_tensor("mh", [128, 128], f32r, kind="ExternalInput")
    mhb = nc.dram_tensor("mhb", [128, 128], bf16, kind="ExternalInput")
    ml = nc.dram_tensor("ml", [128, 128], f32r, kind="ExternalInput")
    wvbd = nc.dram_tensor("wvbd", [128, 128], f32r, kind="ExternalInput")
    identr = nc.dram_tensor("identr", [128, 128], f32r, kind="ExternalInput")
    kb = nc.dram_tensor("kb", [1, BPC * 512], bf16, kind="ExternalInput")
    qm = nc.dram_tensor("qm", [128, BPC * SQT], f32, kind="ExternalInput")
    out = nc.dram_tensor("out", [BPC, SQT, 128, 2048], f32, kind="ExternalOutput")

    Exp = mybir.ActivationFunctionType.Exp
    Copy = mybir.ActivationFunctionType.Copy

    with tile.TileContext(nc) as tc, ExitStack() as ctx:
        singles = ctx.enter_context(tc.tile_pool(name="singles", bufs=1))
        xpool = ctx.enter_context(tc.tile_pool(name="xpool", bufs=2))
        yhpool = ctx.enter_context(tc.tile_pool(name="yhpool", bufs=2))
        ylpool = ctx.enter_context(tc.tile_pool(name="ylpool", bufs=2))
        zpool = ctx.enter_context(tc.tile_pool(name="zpool", bufs=4))
        vpool = ctx.enter_context(tc.tile_pool(name="vpool", bufs=1))
        ppool = ctx.enter_context(tc.tile_pool(name="ppool", bufs=5))
        ptpool = ctx.enter_context(tc.tile_pool(name="ptpool", bufs=5))
        cpool = ctx.enter_context(tc.tile_pool(name="cpool", bufs=2))
        stats = ctx.enter_context(tc.tile_pool(name="stats", bufs=12))
        ps = ctx.enter_context(tc.tile_pool(name="ps", bufs=8, space="PSUM"))

        mh_sb = singles.tile([128, 128], f32r)
        nc.sync.dma_start(out=mh_sb, in_=mh[:, :])
        ml_sb = singles.tile([128, 128], f32r)
        nc.sync.dma_start(out=ml_sb, in_=ml[:, :])
        mhb_sb = singles.tile([128, 128], bf16)
        nc.scalar.dma_start(out=mhb_sb, in_=mhb[:, :])
        wvbd_sb = singles.tile([128, 128], f32r)
        nc.scalar.dma_start(out=wvbd_sb, in_=wvbd[:, :])
        ident_sb = singles.tile([128, 128], f32r)
        nc.scalar.dma_start(out=ident_sb, in_=identr[:, :])
        kb_sb = singles.tile([1, BPC * 512], bf16)
        nc.scalar.dma_start(out=kb_sb, in_=kb[:, :])
        qm_sb = singles.tile([128, BPC * SQT], f32)
        nc.scalar.dma_start(out=qm_sb, in_=qm[:, :])
        ones_sb = singles.tile([1, 128], bf16)
        nc.vector.memset(ones_sb, 1.0)

        for b in range(BPC):
            # first x/yl group goes FIRST so the PE can start immediately;
            # the big yh load only gates scores(0), a few us later.
            xh0 = xpool.tile([128, 2048], f32r, tag="xh")
            nc.sync.dma_start(out=xh0[:, 0:512], in_=xh[b, :, 0:512])
            xl0 = xpool.tile([128, 2048], bf16, tag="xl")
            nc.sync.dma_start(out=xl0, in_=xl[b, :, 0:2048])
            if b == 0:
                nc.vector.tensor_copy(xh0[:1, 512:513], xl0[:1, :1])
            nc.sync.dma_start(out=xh0[:, 512:2048], in_=xh[b, :, 512:2048])
            yl0 = ylpool.tile([128, 2048], bf16, tag="yl")
            # yh resident for the whole batch (scores pass 1/2 rhs + v-proj)
            yh_sb = yhpool.tile([128, CW], f32r)
            if b == 0:
                # stage 1: first yh chunk behind the tiny xh/xl head pieces
                nc.vector.tensor_copy(yh_sb[:1, :1], xh0[:1, :1])
            nc.sync.dma_start(out=yh_sb[:, 0:512], in_=yh[b, :, 0:512])
            if b == 0:
                nc.vector.tensor_copy(yh_sb[:1, 512:513], yh_sb[:1, :1])
            nc.sync.dma_start(out=yh_sb[:, 512:2048], in_=yh[b, :, 512:2048])
            nc.sync.dma_start(out=yl0, in_=yl[b, :, 0:2048])
            for g in range(1, 4):
                if b == 0:
                    # stage 2: remaining yh groups behind yh-g0
                    nc.vector.tensor_copy(
                        yh_sb[:1, g * 2048:g * 2048 + 1],
                        yh_sb[:1, 512:513])
                nc.sync.dma_start(out=yh_sb[:, g * 2048:(g + 1) * 2048],
                                  in_=yh[b, :, g * 2048:(g + 1) * 2048])

            v_sb = vpool.tile([128, CW], f32r)
            # v natural viewed as [128, k(4), 2048]: col k*2048 + d
            v_3d = v_sb.rearrange("p (k d) -> p k d", k=4)

            psS = []
            for i in range(SQT):
                s_tile = ps.tile([128, 512], f32, tag="ps")
                psS.append(s_tile)

            # software-pipelined chunk loop:
            #   stage A(j): load x group, Z-proj 3-pass, split Z -> Zhi/Zlo
            #   stage B(j): scores 3-pass for all 4 sq tiles + v-proj chunk
            zhis, zlos = {}, {}
            yl_tiles = {}
            for j in range(NCH + 1):
                if j < NCH:
                    g = j // 4
                    if j == 0:
                        xh_sb, xl_sb = xh0, xl0
                        yl_tiles[0] = yl0
                    elif j % 4 == 0:
                        xh_sb = xpool.tile([128, 2048], f32r, tag="xh")
                        if b == 0:
                            nc.vector.tensor_copy(xh_sb[:1, :1], yl0[:1, :1])
                        nc.sync.dma_start(out=xh_sb,
                                          in_=xh[b, :, g * 2048:(g + 1) * 2048])
                        xl_sb = xpool.tile([128, 2048], bf16, tag="xl")
                        if b == 0:
                            nc.vector.tensor_copy(xl_sb[:1, :1], yl0[:1, :1])
                        nc.sync.dma_start(out=xl_sb,
                                          in_=xl[b, :, g * 2048:(g + 1) * 2048])
                        yl_sb = ylpool.tile([128, 2048], bf16, tag="yl")
                        if b == 0:
                            nc.vector.tensor_copy(yl_sb[:1, :1], yl0[:1, :1])
                        nc.sync.dma_start(out=yl_sb,
                                          in_=yl[b, :, g * 2048:(g + 1) * 2048])
                        yl_tiles[g] = yl_sb
                    u = (j % 4) * 512
                    psz = ps.tile([128, 512], f32, tag="ps")
                    nc.tensor.matmul(psz, mh_sb, xh_sb[:, u:u + 512],
                                     start=True, stop=False)
                    nc.tensor.matmul(psz, ml_sb, xh_sb[:, u:u + 512],
                                     start=False, stop=False)
                    nc.tensor.matmul(psz, mhb_sb, xl_sb[:, u:u + 512],
                                     start=False, stop=True)
                    zhi = zpool.tile([128, 512], f32r, tag="zhi")
                    nc.scalar.copy(zhi, psz)
                    zlo = zpool.tile([128, 512], f32r, tag="zlo")
                    nc.vector.tensor_sub(zlo, psz, zhi)
                    zhb = zpool.tile([128, 512], bf16, tag="zhb")
                    nc.vector.tensor_copy(zhb, zhi)
                    zhis[j], zlos[j] = (zhi, zhb), zlo

                jj = j - 1
                if jj < 0:
                    continue
                (zhi, zhb), zlo = zhis.pop(jj), zlos.pop(jj)
                yhj = yh_sb[:, jj * 512:(jj + 1) * 512]
                ylj = yl_tiles[jj // 4][:, (jj % 4) * 512:(jj % 4 + 1) * 512]
                for i in range(SQT):
                    c0 = i * 128
                    nc.tensor.matmul(psS[i], zhi[:, c0:c0 + 128], yhj,
                                     start=(jj == 0), stop=False)
                    nc.tensor.matmul(psS[i], zlo[:, c0:c0 + 128], yhj,
                                     start=False, stop=False)
                    nc.tensor.matmul(psS[i], zhb[:, c0:c0 + 128], ylj,
                                     start=False, stop=False)
                if jj < 12:
                    # v-proj for chunk jj: 4 t-chunk blocks into one psum tile
                    psv = ps.tile([128, 512], f32, tag="ps")
                    for k in range(4):
                        nc.tensor.matmul(psv[:, k * 128:(k + 1) * 128],
                                         yhj[:, k * 128:(k + 1) * 128],
                                         wvbd_sb, start=True, stop=True)
                    # psv[:, (k,c)] -> v_sb[:, k*2048 + jj*128 + c]
                    nc.vector.tensor_copy(v_3d[:, :, jj * 128:(jj + 1) * 128],
                                          psv.rearrange("p (k c) -> p k c", k=4))

            # ---- finish scores: kbias rank-1, then softmax per sq tile ----
            p_tiles = []
            scale_tiles = []
            for i in range(SQT):
                nc.tensor.matmul(psS[i], ones_sb, kb_sb[:, b * 512:(b + 1) * 512],
                                 start=False, stop=True)
                mx = stats.tile([128, 1], f32, tag="mx")
                nc.vector.reduce_max(out=mx, in_=psS[i], axis=mybir.AxisListType.X)
                negmx = stats.tile([128, 1], f32, tag="negmx")
                nc.vector.tensor_scalar_mul(negmx, mx, -1.0)
                p_sb = ppool.tile([128, 512], f32r, tag="p")
                sumexp = stats.tile([128, 1], f32, tag="sumexp")
                nc.scalar.activation(p_sb, psS[i], Exp, bias=negmx, scale=1.0,
                                     accum_out=sumexp)
                rsum = stats.tile([128, 1], f32, tag="rsum")
                nc.vector.reciprocal(rsum, sumexp)
                scale_i = stats.tile([128, 1], f32, tag="scale")
                nc.vector.tensor_mul(scale_i, rsum,
                                     qm_sb[:, b * SQT + i: b * SQT + i + 1])
                p_tiles.append(p_sb)
                scale_tiles.append(scale_i)

            # ---- deferred v-proj chunks (fill the PE while the softmax
            # chain for the last sq tile drains on DVE/ACT) ----
            for jj in range(12, NCH):
                yhj = yh_sb[:, jj * 512:(jj + 1) * 512]
                psv = ps.tile([128, 512], f32, tag="ps")
                for kk in range(4):
                    nc.tensor.matmul(psv[:, kk * 128:(kk + 1) * 128],
                                     yhj[:, kk * 128:(kk + 1) * 128],
                                     wvbd_sb, start=True, stop=True)
                nc.vector.tensor_copy(v_3d[:, :, jj * 128:(jj + 1) * 128],
                                      psv.rearrange("p (k c) -> p k c", k=4))

            # ---- P^T transposes interleaved with AV(i=0) k-slices: real
            # matmuls between transpose groups keep the HAM clock warm
            # (transpose-mode doesn't count as PE-busy) ----
            pt_tiles = []
            ctx0 = cpool.tile([128, 2048], f32)
            psc0 = []
            for _n in range(4):
                pc = ps.tile([128, 512], f32, tag="ps")
                psc0.append(pc)
            for k in range(SQT):            # sk-chunk
                pspt = ps.tile([128, 512], f32r, tag="ps")
                for i in range(SQT):
                    nc.tensor.transpose(
                        pspt[:, i * 128:(i + 1) * 128],
                        p_tiles[i][:, k * 128:(k + 1) * 128],
                        ident_sb)
                pt_sb = ptpool.tile([128, 512], f32r, tag="pt")
                nc.vector.tensor_copy(pt_sb, pspt)
                pt_tiles.append(pt_sb)
                for n in range(4):
                    nc.tensor.matmul(
                        psc0[n],
                        pt_tiles[k][:, 0:128],
                        v_3d[:, k, n * 512:(n + 1) * 512],
                        start=(k == 0), stop=(k == SQT - 1))
            for n in range(4):
                if n % 2 == 0:
                    nc.scalar.activation(ctx0[:, n * 512:(n + 1) * 512],
                                         psc0[n], Copy, bias=0.0,
                                         scale=scale_tiles[0])
                else:
                    nc.vector.tensor_scalar_mul(
                        ctx0[:, n * 512:(n + 1) * 512], psc0[n],
                        scale_tiles[0])
                nc.sync.dma_start(out=out[b, 0, :, n * 512:(n + 1) * 512],
                                  in_=ctx0[:, n * 512:(n + 1) * 512])

            # ---- ctx = P^T.T @ v (fp32r), normalize+Qmask fused on copy ----
            for i in range(1, SQT):
                ctx_sb = cpool.tile([128, 2048], f32)
                for n in range(4):
                    psc = ps.tile([128, 512], f32, tag="ps")
                    for k in range(SQT):
                        nc.tensor.matmul(
                            psc,
                            pt_tiles[k][:, i * 128:(i + 1) * 128],
                            v_3d[:, k, n * 512:(n + 1) * 512],
                            start=(k == 0), stop=(k == SQT - 1))
                    if n % 2 == 0:
                        nc.scalar.activation(ctx_sb[:, n * 512:(n + 1) * 512],
                                             psc, Copy, bias=0.0,
                                             scale=scale_tiles[i])
                    else:
                        nc.vector.tensor_scalar_mul(
                            ctx_sb[:, n * 512:(n + 1) * 512], psc,
                            scale_tiles[i])
                    nc.sync.dma_start(out=out[b, i, :, n * 512:(n + 1) * 512],
                                      in_=ctx_sb[:, n * 512:(n + 1) * 512])

    nc.compile()
    return nc


def _cat_w(wr, wi):
    """[[Wr, Wi], [-Wi, Wr]] : (e_cat 64) x (f_cat 64)."""
    top = np.concatenate([wr, wi], axis=1)
    bot = np.concatenate([-wi, wr], axis=1)
    return np.concatenate([top, bot], axis=0)


def _bd(w):
    z = np.zeros_like(w)
    return np.block([[w, z], [z, w]]).astype(np.float32)


def _tf32(x):
    x = np.ascontiguousarray(x, np.float32)
    xi = x.view(np.uint32)
    lsb = (xi >> 13) & np.uint32(1)
    rounded = (xi + np.uint32(0x0FFF) + lsb) & np.uint32(0xFFFFE000)
    return rounded.view(np.float32)


def _split(x):
    hi = _tf32(x)
    lo = _tf32(np.asarray(x, np.float32) - hi)
    return hi, lo


def _prep(inputs):
    """Pure layout transforms + O(weight) algebra on host."""
    Qr, Qi = np.asarray(inputs['Q_r']), np.asarray(inputs['Q_i'])
    KVr, KVi = np.asarray(inputs['KV_r']), np.asarray(inputs['KV_i'])
    Km, Qm = np.asarray(inputs['K_mask']), np.asarray(inputs['Q_mask'])

    X = np.concatenate([Qr, Qi], axis=-1)     # [B, S, 32, 64]
    Y = np.concatenate([KVr, KVi], axis=-1)
    # X^T layout: [B, 128, 16*512] with partition p of chunk j = row j*128+p
    # of the flattened (x*64 + c) axis.
    def to_xt(A):
        At = A.transpose(0, 2, 3, 1).reshape(B, 2048, S)        # [B, (x c), S]
        At = At.reshape(B, NCH, 128, S).transpose(0, 2, 1, 3)   # [B, 128, 16, S]
        return np.ascontiguousarray(At.reshape(B, 128, NCH * S), np.float32)

    xh, xl_ = _split(to_xt(X))
    xl = xl_.astype(ml_dtypes.bfloat16)
    yh, yl_ = _split(to_xt(Y))
    yl = yl_.astype(ml_dtypes.bfloat16)

    Wq = _cat_w(np.asarray(inputs['Wq_r']), np.asarray(inputs['Wq_i']))
    Wk = _cat_w(np.asarray(inputs['Wk_r']), np.asarray(inputs['Wk_i']))
    Wv = _cat_w(np.asarray(inputs['Wv_r']), np.asarray(inputs['Wv_i']))
    M2 = (Wq.astype(np.float64) @ Wk.astype(np.float64).T).astype(np.float32)
    mh_, ml_ = _split(_bd(M2))
    mhb_ = mh_.astype(ml_dtypes.bfloat16)
    wvbd = _tf32(_bd(Wv.astype(np.float32)))
    ident = np.eye(128, dtype=np.float32)

    kbias = ((1.0 - Km) * -100000.0).astype(ml_dtypes.bfloat16)  # [B, S]
    in_maps = []
    for c in range(NCORES):
        bs = slice(c * BPC, (c + 1) * BPC)
        qm_c = np.ascontiguousarray(
            Qm[bs].reshape(BPC, SQT, 128).transpose(2, 0, 1)
            .reshape(128, BPC * SQT), np.float32)
        in_maps.append({
            "xh": xh[bs], "xl": xl[bs], "yh": yh[bs], "yl": yl[bs],
            "mh": mh_, "ml": ml_, "mhb": mhb_, "wvbd": wvbd, "identr": ident,
            "kb": np.ascontiguousarray(kbias[bs].reshape(1, BPC * 512)),
            "qm": qm_c,
        })
    return in_maps


def kernel(_trace=False, _tmpdir=None, **inputs):
    global LAST_EXEC_NS, _NC_CACHE
    in_maps = _prep(inputs)
    if _NC_CACHE is None:
        _NC_CACHE = build_nc()
    res = run_bass_kernel_spmd(_NC_CACHE, in_maps, core_ids=list(range(NCORES)),
                               trace=_trace, tmpdir=_tmpdir)
    LAST_EXEC_NS = res.exec_time_ns
    outs = [res.results[c]["out"] for c in range(NCORES)]
    ctx = np.concatenate(outs, axis=0)          # [B, 4, 128, 2048]
    ctx = ctx.reshape(B, S, 32, 2, 32)          # [B, S, x, (r|i), f]
    return (ctx[..., 0, :] + 1j * ctx[..., 1, :]).astype(np.complex64)



# revision 2
# speedup vs baseline: 1.7556x; 1.7556x over previous
"""Complex-valued attention (nn_Attention_1) on 8 Trainium2 NeuronCores.

Math (per batch b):
  q = X @ Wq_cat, k = Y @ Wk_cat, v = Y @ Wv_cat  with X=[Q_r|Q_i], Y=[KV_r|KV_i]
  scores = qr kr^T + qi ki^T  ==  sum_x X_x (Wq_cat Wk_cat^T) Y_x^T
  probs = softmax(scores + kmask_bias); ctx = probs @ v * Q_mask
Sharding: data-parallel over B=16 -> 2 batches per core, no cross-core comm.

Precision scheme: all matmuls single-pass fp32r (TF32-grade operands, fp32 PSUM
accumulation).  The softmax is argmax-sharp (score std ~88) but its output is a
continuous function of the scores: per-row sensitivity is bounded by
0.25*|dS|*|v1-v2| even at exact ties, and exponentially suppressed elsewhere.
Simulated end-to-end L2 rel err of this scheme (incl. bf16 output) is 4.6e-3
vs the 2e-2 gate.  Output is written as bf16 (halves output DMA); the host
converts to complex64.
Normalization (1/sumexp) and Q_mask are fused into the ctx PSUM->SBUF copy.
"""
import sys
sys.path.insert(0, '/opt/trn_rl_repo')
import numpy as np
import ml_dtypes
from contextlib import ExitStack

import concourse.bass as bass
from concourse import bacc
import concourse.mybir as mybir
import concourse.tile as tile
from concourse.bass_utils import run_bass_kernel_spmd

B, S, E = 16, 512, 32
NCORES = 8
BPC = B // NCORES           # batches per core
NCH = 16                    # 128-row chunks of the 2048-wide (x, e-cat) axis
SQT = S // 128              # 4 sq tiles per batch

f32 = mybir.dt.float32
f32r = mybir.dt.float32r
bf16 = mybir.dt.bfloat16

LAST_EXEC_NS = None
_NC_CACHE = None


def build_nc():
    nc = bacc.Bacc()
    CW = NCH * 512
    xh = nc.dram_tensor("xh", [BPC, 128, CW], f32r, kind="ExternalInput")
    yh = nc.dram_tensor("yh", [BPC, 128, CW], f32r, kind="ExternalInput")
    mh = nc.dram_tensor("mh", [128, 128], f32r, kind="ExternalInput")
    wvbd = nc.dram_tensor("wvbd", [128, 128], f32r, kind="ExternalInput")
    identr = nc.dram_tensor("identr", [128, 128], f32r, kind="ExternalInput")
    kb = nc.dram_tensor("kb", [1, BPC * 512], bf16, kind="ExternalInput")
    qm = nc.dram_tensor("qm", [128, BPC * SQT], f32, kind="ExternalInput")
    out = nc.dram_tensor("out", [BPC, SQT, 128, 2048], bf16, kind="ExternalOutput")

    Exp = mybir.ActivationFunctionType.Exp
    Copy = mybir.ActivationFunctionType.Copy

    with tile.TileContext(nc) as tc, ExitStack() as ctx:
        singles = ctx.enter_context(tc.tile_pool(name="singles", bufs=1))
        xpool = ctx.enter_context(tc.tile_pool(name="xpool", bufs=2))
        yhpool = ctx.enter_context(tc.tile_pool(name="yhpool", bufs=2))
        zpool = ctx.enter_context(tc.tile_pool(name="zpool", bufs=3))
        vpool = ctx.enter_context(tc.tile_pool(name="vpool", bufs=1))
        ppool = ctx.enter_context(tc.tile_pool(name="ppool", bufs=5))
        ptpool = ctx.enter_context(tc.tile_pool(name="ptpool", bufs=5))
        cpool = ctx.enter_context(tc.tile_pool(name="cpool", bufs=2))
        stats = ctx.enter_context(tc.tile_pool(name="stats", bufs=12))
        ps = ctx.enter_context(tc.tile_pool(name="ps", bufs=8, space="PSUM"))

        mh_sb = singles.tile([128, 128], f32r)
        nc.sync.dma_start(out=mh_sb, in_=mh[:, :])
        wvbd_sb = singles.tile([128, 128], f32r)
        nc.scalar.dma_start(out=wvbd_sb, in_=wvbd[:, :])
        ident_sb = singles.tile([128, 128], f32r)
        nc.scalar.dma_start(out=ident_sb, in_=identr[:, :])
        kb_sb = singles.tile([1, BPC * 512], bf16)
        nc.scalar.dma_start(out=kb_sb, in_=kb[:, :])
        qm_sb = singles.tile([128, BPC * SQT], f32)
        nc.scalar.dma_start(out=qm_sb, in_=qm[:, :])
        ones_sb = singles.tile([1, 128], bf16)
        nc.vector.memset(ones_sb, 1.0)

        for b in range(BPC):
            # first xh chunk goes FIRST so the PE can start immediately;
            # yh chunk 0 right behind it (gates scores(0)), then the rest.
            xh0 = xpool.tile([128, 2048], f32r, tag="xh")
            nc.sync.dma_start(out=xh0[:, 0:512], in_=xh[b, :, 0:512])
            yh_sb = yhpool.tile([128, CW], f32r)
            if b == 0:
                # stage: yh chunk 0 behind xh chunk 0
                nc.vector.tensor_copy(yh_sb[:1, :1], xh0[:1, :1])
            nc.sync.dma_start(out=yh_sb[:, 0:512], in_=yh[b, :, 0:512])
            if b == 0:
                nc.vector.tensor_copy(xh0[:1, 512:513], yh_sb[:1, :1])
            nc.sync.dma_start(out=xh0[:, 512:2048], in_=xh[b, :, 512:2048])
            if b == 0:
                nc.vector.tensor_copy(yh_sb[:1, 512:513], xh0[:1, 512:513])
            nc.sync.dma_start(out=yh_sb[:, 512:2048], in_=yh[b, :, 512:2048])
            for g in range(1, 4):
                if b == 0:
                    # remaining yh groups staged behind group g-1
                    nc.vector.tensor_copy(
                        yh_sb[:1, g * 2048:g * 2048 + 1],
                        yh_sb[:1, 512:513])
                nc.sync.dma_start(out=yh_sb[:, g * 2048:(g + 1) * 2048],
                                  in_=yh[b, :, g * 2048:(g + 1) * 2048])

            v_sb = vpool.tile([128, CW], f32r)
            # v natural viewed as [128, k(4), 2048]: col k*2048 + d
            v_3d = v_sb.rearrange("p (k d) -> p k d", k=4)

            psS = []
            for i in range(SQT):
                s_tile = ps.tile([128, 512], f32, tag="ps")
                psS.append(s_tile)

            # software-pipelined chunk loop:
            #   stage A(j): load x group, Z-proj, copy Z -> zhi (f32r)
            #   stage B(j): scores for all 4 sq tiles + v-proj chunk
            zhis = {}
            for j in range(NCH + 1):
                if j < NCH:
                    g = j // 4
                    if j == 0:
                        xh_sb = xh0
                    elif j % 4 == 0:
                        xh_sb = xpool.tile([128, 2048], f32r, tag="xh")
                        if b == 0:
                            nc.vector.tensor_copy(xh_sb[:1, :1], yh_sb[:1, :1])
                        nc.sync.dma_start(out=xh_sb,
                                          in_=xh[b, :, g * 2048:(g + 1) * 2048])
                    u = (j % 4) * 512
                    psz = ps.tile([128, 512], f32, tag="ps")
                    nc.tensor.matmul(psz, mh_sb, xh_sb[:, u:u + 512],
                                     start=True, stop=True)
                    zhi = zpool.tile([128, 512], f32r, tag="zhi")
                    nc.scalar.copy(zhi, psz)
                    zhis[j] = zhi

                jj = j - 1
                if jj < 0:
                    continue
                zhi = zhis.pop(jj)
                yhj = yh_sb[:, jj * 512:(jj + 1) * 512]
                for i in range(SQT):
                    c0 = i * 128
                    nc.tensor.matmul(psS[i], zhi[:, c0:c0 + 128], yhj,
                                     start=(jj == 0), stop=False)
                if jj < 12:
                    # v-proj for chunk jj: 4 t-chunk blocks into one psum tile
                    psv = ps.tile([128, 512], f32, tag="ps")
                    for k in range(4):
                        nc.tensor.matmul(psv[:, k * 128:(k + 1) * 128],
                                         yhj[:, k * 128:(k + 1) * 128],
                                         wvbd_sb, start=True, stop=True)
                    # psv[:, (k,c)] -> v_sb[:, k*2048 + jj*128 + c]
                    nc.vector.tensor_copy(v_3d[:, :, jj * 128:(jj + 1) * 128],
                                          psv.rearrange("p (k c) -> p k c", k=4))

            # ---- finish scores: kbias rank-1, then softmax per sq tile ----
            p_tiles = []
            scale_tiles = []
            for i in range(SQT):
                nc.tensor.matmul(psS[i], ones_sb, kb_sb[:, b * 512:(b + 1) * 512],
                                 start=False, stop=True)
                mx = stats.tile([128, 1], f32, tag="mx")
                nc.vector.reduce_max(out=mx, in_=psS[i], axis=mybir.AxisListType.X)
                negmx = stats.tile([128, 1], f32, tag="negmx")
                nc.vector.tensor_scalar_mul(negmx, mx, -1.0)
                p_sb = ppool.tile([128, 512], f32r, tag="p")
                sumexp = stats.tile([128, 1], f32, tag="sumexp")
                nc.scalar.activation(p_sb, psS[i], Exp, bias=negmx, scale=1.0,
                                     accum_out=sumexp)
                rsum = stats.tile([128, 1], f32, tag="rsum")
                nc.vector.reciprocal(rsum, sumexp)
                scale_i = stats.tile([128, 1], f32, tag="scale")
                nc.vector.tensor_mul(scale_i, rsum,
                                     qm_sb[:, b * SQT + i: b * SQT + i + 1])
                p_tiles.append(p_sb)
                scale_tiles.append(scale_i)

            # ---- deferred v-proj chunks (fill the PE while the softmax
            # chain for the last sq tile drains on DVE/ACT) ----
            for jj in range(12, NCH):
                yhj = yh_sb[:, jj * 512:(jj + 1) * 512]
                psv = ps.tile([128, 512], f32, tag="ps")
                for kk in range(4):
                    nc.tensor.matmul(psv[:, kk * 128:(kk + 1) * 128],
                                     yhj[:, kk * 128:(kk + 1) * 128],
                                     wvbd_sb, start=True, stop=True)
                nc.vector.tensor_copy(v_3d[:, :, jj * 128:(jj + 1) * 128],
                                      psv.rearrange("p (k c) -> p k c", k=4))

            # ---- P^T transposes interleaved with AV(i=0) k-slices: real
            # matmuls between transpose groups keep the HAM clock warm
            # (transpose-mode doesn't count as PE-busy) ----
            pt_tiles = []
            ctx0 = cpool.tile([128, 2048], bf16)
            psc0 = []
            for _n in range(4):
                pc = ps.tile([128, 512], f32, tag="ps")
                psc0.append(pc)
            for k in range(SQT):            # sk-chunk
                pspt = ps.tile([128, 512], f32r, tag="ps")
                for i in range(SQT):
                    nc.tensor.transpose(
                        pspt[:, i * 128:(i + 1) * 128],
                        p_tiles[i][:, k * 128:(k + 1) * 128],
                        ident_sb)
                pt_sb = ptpool.tile([128, 512], f32r, tag="pt")
                nc.vector.tensor_copy(pt_sb, pspt)
                pt_tiles.append(pt_sb)
                for n in range(4):
                    nc.tensor.matmul(
                        psc0[n],
                        pt_tiles[k][:, 0:128],
                        v_3d[:, k, n * 512:(n + 1) * 512],
                        start=(k == 0), stop=(k == SQT - 1))
            for n in range(4):
                if n % 2 == 0:
                    nc.scalar.activation(ctx0[:, n * 512:(n + 1) * 512],
                                         psc0[n], Copy, bias=0.0,
                                         scale=scale_tiles[0])
                else:
                    nc.vector.tensor_scalar_mul(
                        ctx0[:, n * 512:(n + 1) * 512], psc0[n],
                        scale_tiles[0])
            nc.sync.dma_start(out=out[b, 0, :, :], in_=ctx0)

            # ---- ctx = P^T.T @ v (fp32r), normalize+Qmask fused on copy ----
            for i in range(1, SQT):
                ctx_sb = cpool.tile([128, 2048], bf16)
                for n in range(4):
                    psc = ps.tile([128, 512], f32, tag="ps")
                    for k in range(SQT):
                        nc.tensor.matmul(
                            psc,
                            pt_tiles[k][:, i * 128:(i + 1) * 128],
                            v_3d[:, k, n * 512:(n + 1) * 512],
                            start=(k == 0), stop=(k == SQT - 1))
                    if n % 2 == 0:
                        nc.scalar.activation(ctx_sb[:, n * 512:(n + 1) * 512],
                                             psc, Copy, bias=0.0,
                                             scale=scale_tiles[i])
                    else:
                        nc.vector.tensor_scalar_mul(
                            ctx_sb[:, n * 512:(n + 1) * 512], psc,
                            scale_tiles[i])
                nc.sync.dma_start(out=out[b, i, :, :], in_=ctx_sb)

    nc.compile()
    return nc


def _cat_w(wr, wi):
    """[[Wr, Wi], [-Wi, Wr]] : (e_cat 64) x (f_cat 64)."""
    top = np.concatenate([wr, wi], axis=1)
    bot = np.concatenate([-wi, wr], axis=1)
    return np.concatenate([top, bot], axis=0)


def _bd(w):
    z = np.zeros_like(w)
    return np.block([[w, z], [z, w]]).astype(np.float32)


def _prep(inputs):
    """Pure layout transforms + O(weight) algebra on host."""
    Qr, Qi = np.asarray(inputs['Q_r']), np.asarray(inputs['Q_i'])
    KVr, KVi = np.asarray(inputs['KV_r']), np.asarray(inputs['KV_i'])
    Km, Qm = np.asarray(inputs['K_mask']), np.asarray(inputs['Q_mask'])

    X = np.concatenate([Qr, Qi], axis=-1)     # [B, S, 32, 64]
    Y = np.concatenate([KVr, KVi], axis=-1)
    # X^T layout: [B, 128, 16*512] with partition p of chunk j = row j*128+p
    # of the flattened (x*64 + c) axis.
    def to_xt(A):
        At = A.transpose(0, 2, 3, 1).reshape(B, 2048, S)        # [B, (x c), S]
        At = At.reshape(B, NCH, 128, S).transpose(0, 2, 1, 3)   # [B, 128, 16, S]
        return np.ascontiguousarray(At.reshape(B, 128, NCH * S), np.float32)

    xh = to_xt(X)
    yh = to_xt(Y)

    Wq = _cat_w(np.asarray(inputs['Wq_r']), np.asarray(inputs['Wq_i']))
    Wk = _cat_w(np.asarray(inputs['Wk_r']), np.asarray(inputs['Wk_i']))
    Wv = _cat_w(np.asarray(inputs['Wv_r']), np.asarray(inputs['Wv_i']))
    M2 = (Wq.astype(np.float64) @ Wk.astype(np.float64).T).astype(np.float32)
    mh_ = _bd(M2)
    wvbd = _bd(Wv.astype(np.float32))
    ident = np.eye(128, dtype=np.float32)

    kbias = ((1.0 - Km) * -100000.0).astype(ml_dtypes.bfloat16)  # [B, S]
    in_maps = []
    for c in range(NCORES):
        bs = slice(c * BPC, (c + 1) * BPC)
        qm_c = np.ascontiguousarray(
            Qm[bs].reshape(BPC, SQT, 128).transpose(2, 0, 1)
            .reshape(128, BPC * SQT), np.float32)
        in_maps.append({
            "xh": xh[bs], "yh": yh[bs],
            "mh": mh_, "wvbd": wvbd, "identr": ident,
            "kb": np.ascontiguousarray(kbias[bs].reshape(1, BPC * 512)),
            "qm": qm_c,
        })
    return in_maps


def kernel(_trace=False, _tmpdir=None, **inputs):
    global LAST_EXEC_NS, _NC_CACHE
    in_maps = _prep(inputs)
    if _NC_CACHE is None:
        _NC_CACHE = build_nc()
    res = run_bass_kernel_spmd(_NC_CACHE, in_maps, core_ids=list(range(NCORES)),
                               trace=_trace, tmpdir=_tmpdir)
    LAST_EXEC_NS = res.exec_time_ns
    outs = [np.asarray(res.results[c]["out"], dtype=np.float32)
            for c in range(NCORES)]
    ctx = np.concatenate(outs, axis=0)          # [B, 4, 128, 2048]
    ctx = ctx.reshape(B, S, 32, 2, 32)          # [B, S, x, (r|i), f]
    return (ctx[..., 0, :] + 1j * ctx[..., 1, :]).astype(np.complex64)
